# revision 23
# baseline (speedup 1.0000x reference)
"""Multi-head attention (B=2, S=2048, D=4096, H=32, HD=128) on 8 NeuronCores.

Tensor-parallel over heads: core c owns heads 4c..4c+3 (512 hidden dims).
All matmuls in bf16 (f32 PSUM accumulation); rel-err ~5e-3 vs fp32 reference.

I/O is minimized with on-device collectives:
  - Each core receives ONLY its own 512-token x^T tile (4MB bf16); the full
    x^T is assembled on-device with two AllGathers split by TOKEN halves:
    every 128-token subtile of half 0 is complete after the first gather,
    so phase A sweeps (batch 0 half 0, batch 1 half 0, batch 0 half 1,
    batch 1 half 1) and the PE never waits for the second gather.
  - Rope tables and the causal band mask are embedded in the NEFF as Const
    tensors (they are deterministic functions of the rope/mask inputs; the
    build cache is keyed on their digest, so different values rebuild).
  - The RowParallel all-reduce after wo runs on-device as 4 chunked
    ReduceScatters (bf16 when OUT_BF16, else f32); each core outputs a
    disjoint [512, 4096] slice and the host only reassembles (no
    host-side summation).

Per core, one NEFF with three phases:
  A: fused Q/K/V projections from one pass over the gathered x^T (bf16).
     RoPE on DVE in [t, hd] layout with host-permuted weight columns
     (rope pairs split into contiguous r/i halves); Q/K transposed per
     head on the PE (bf16, cheap) and staged to DRAM; V kept natural
     [t, hd] and staged to DRAM.
  B: attention with TRANSPOSED scores S^T[k, q] so no P transposes are
     needed: S^T = K_chunk^T-as-stationary @ Q^T, exp on ACT, PV and the
     softmax denominator (ones-column matmul) accumulate on the PE, and
     normalization uses an outer-product broadcast of 1/z. Causal masking
     multiplies the 4 diagonal k-tiles by a 0/1 band on DVE post-exp
     (scores are small enough that exp never overflows); fully masked
     k-tiles are skipped.  O^T stays resident in SBUF.
  C: output projection from resident O^T and wo; per-core partial written
     as [T, D] to DRAM and reduce-scattered in 4 chunks.
"""
import sys
sys.path.insert(0, '/opt/trn_rl_repo')

import hashlib
import math
import numpy as np
import ml_dtypes

import concourse.mybir as mybir
import concourse.tile as tile
from concourse import bacc
from concourse.bass_utils import run_bass_kernel_spmd
from concourse.masks import make_identity

F32 = mybir.dt.float32
BF16 = mybir.dt.bfloat16

B, S, D = 2, 2048, 4096
H, HD = 32, 128
NCORES = 8
HPC = H // NCORES          # heads per core = 4
DPC = HPC * HD             # hidden dims per core = 512
T = B * S                  # 4096 flattened tokens
NT = T // 512              # 8 x-tiles of 512 tokens
QG = S // 512              # 4 query groups per batch
RS_CHUNKS = 4              # output ReduceScatter granularity
OUT_BF16 = True            # bf16 partials + reduce-scatter (halves output
                           # bytes; host upcasts); f32 if False
GROUPS = [list(range(NCORES))]
ODT = BF16 if OUT_BF16 else F32


def _rope_tables(freqs_cos, freqs_sin):
    """[S, 64] -> [128, S//128, HPC*64] bf16, cos/sin repeated per head."""
    bf = ml_dtypes.bfloat16

    def ttile(a):
        rep = np.tile(a.astype(np.float32), (1, HPC))
        return np.ascontiguousarray(
            rep.reshape(S // 128, 128, HPC * HD // 2)
            .transpose(1, 0, 2)).astype(bf)

    return ttile(freqs_cos), ttile(freqs_sin)


def build_nc(cst_arr, snt_arr, mask2d=None, reps=1, phases=("a", "b", "c")):
    """mask2d=None -> causal fast path (0/1 band multiply); else general
    [S, S] additive mask applied via identity matmul."""
    causal = mask2d is None
    bf = ml_dtypes.bfloat16
    nc = bacc.Bacc("TRN2", target_bir_lowering=False, debug=False,
                   num_devices=NCORES)

    xtc_d = nc.dram_tensor("xtc", [2, 128, 32, 256], BF16,
                           kind="ExternalInput").ap()
    wq_d = nc.dram_tensor("wq3", [128, 32, DPC], BF16,
                          kind="ExternalInput").ap()
    wk_d = nc.dram_tensor("wk3", [128, 32, DPC], BF16,
                          kind="ExternalInput").ap()
    wv_d = nc.dram_tensor("wv3", [128, 32, DPC], BF16,
                          kind="ExternalInput").ap()
    wo_d = nc.dram_tensor("wo3", [128, HPC, D], BF16,
                          kind="ExternalInput").ap()
    # rope tables in [t, h*pair] layout (repeated per head), tiled by
    # 128-token subtile; the 1/sqrt(HD) query scale is folded into wq.
    cs_d = nc.inline_tensor(cst_arr, "cst").ap()
    sn_d = nc.inline_tensor(snt_arr, "snt").ap()
    if causal:
        # 0/1 keep-mask band for transposed scores, applied post-exp on DVE:
        # band[p, r, j] = 0 if r*128 + p > j else 1  (the 4 diagonal-band
        # k-tiles of any 512-wide q group)
        p_i = np.arange(128)[:, None, None]
        r_i = np.arange(4)[None, :, None]
        j_i = np.arange(512)[None, None, :]
        band = np.where(r_i * 128 + p_i > j_i, 0.0, 1.0).astype(bf)
        mb_d = nc.inline_tensor(band, "maskb").ap()
    else:
        # full transposed additive mask, [k-part, kt, q]
        mt = np.ascontiguousarray(
            mask2d.T.reshape(S // 128, 128, S).transpose(1, 0, 2)).astype(bf)
        mb_d = nc.inline_tensor(mt, "maskt").ap()

    outs_d = nc.dram_tensor("outs", [T // NCORES, D], ODT,
                            kind="ExternalOutput").ap()

    # DRAM scratch.  Collective ins/outs must be Internal (NEFF-static
    # addresses), so x^T bounces through xtin_* and the output partial
    # lives in part_d.
    xtin0 = nc.dram_tensor("xtin0", [128, 32, 256], BF16).ap()
    xtin1 = nc.dram_tensor("xtin1", [128, 32, 256], BF16).ap()
    xt_h0 = nc.dram_tensor("xt_h0", [NT, 128, 32, 256], BF16,
                           addr_space="Shared").ap()
    xt_h1 = nc.dram_tensor("xt_h1", [NT, 128, 32, 256], BF16,
                           addr_space="Shared").ap()
    part_d = nc.dram_tensor("part", [T, D], ODT).ap()
    rso_d = nc.dram_tensor("rso", [RS_CHUNKS, T // NCORES // RS_CHUNKS, D],
                           ODT).ap()
    # Q^T/K^T staged per x-tile as [tile][part][head][512t], V natural rows
    qt_d = nc.dram_tensor("qt_s", [NT, 128, HPC, 512], BF16).ap()
    kt_d = nc.dram_tensor("kt_s", [NT, 128, HPC, 512], BF16).ap()
    v_d = nc.dram_tensor("v_s", [B, S // 128, 128, DPC], BF16).ap()

    with tile.TileContext(nc) as tc:
        # --- gather x^T from all cores, split by token halves so phase A
        # can run an entire half-sweep after the first gather ---
        nc.sync.dma_start(xtin0, xtc_d[0])
        nc.sync.dma_start(xtin1, xtc_d[1])
        nc.gpsimd.collective_compute(
            "AllGather", mybir.AluOpType.bypass, replica_groups=GROUPS,
            ins=[xtin0], outs=[xt_h0])
        nc.gpsimd.collective_compute(
            "AllGather", mybir.AluOpType.bypass, replica_groups=GROUPS,
            ins=[xtin1], outs=[xt_h1])

        with tc.tile_pool(name="const", bufs=1) as constp:
            ident_bf = constp.tile([128, 128], BF16)
            make_identity(nc, ident_bf)
            ones_bf = constp.tile([128, 128], BF16, tag="ones")
            nc.vector.memset(ones_bf, 1.0)
            cs_sb = constp.tile([128, S // 128, 256], BF16, tag="cs")
            sn_sb = constp.tile([128, S // 128, 256], BF16, tag="sn")

            for _rep in range(reps):
                # ---------------- Phase A: QKV projections + rope ------------
                if "a" in phases:
                    with tc.tile_pool(name="aw", bufs=1) as wp, \
                         tc.tile_pool(name="ax", bufs=2) as xp, \
                         tc.tile_pool(name="as", bufs=2) as sp, \
                         tc.tile_pool(name="art", bufs=2) as rtp, \
                         tc.tile_pool(name="aqkps", bufs=3,
                                      space="PSUM") as pp, \
                         tc.tile_pool(name="avps", bufs=2,
                                      space="PSUM") as vpp, \
                         tc.tile_pool(name="atp", bufs=3,
                                      space="PSUM") as tpp:
                        wq_sb = wp.tile([128, 32, DPC], BF16, tag="wq")
                        wk_sb = wp.tile([128, 32, DPC], BF16, tag="wk")
                        wv_sb = wp.tile([128, 32, DPC], BF16, tag="wv")

                        xt_h = (xt_h0, xt_h1)

                        def load_x(x_sb, half, tc_i, first=False):
                            # [128, 32, 256]: this tile's tokens for one half
                            if first:
                                chunks = [(0, 2), (2, 8), (8, 16), (16, 32)]
                            else:
                                chunks = [(0, 32)]
                            for lo, hi in chunks:
                                nc.sync.dma_start(x_sb[:, lo:hi],
                                                  xt_h[half][tc_i][:, lo:hi])

                        # DMA issue order puts the first matmul group's
                        # operands (wq kt0-7, x tile 0) and the rope tables
                        # at the head of the queue; the remaining weights
                        # stream in behind them.
                        cA = slice(0, 2)
                        cB = slice(2, 8)
                        nc.sync.dma_start(wq_sb[:, cA], wq_d[:, cA])
                        x0_sb = xp.tile([128, 32, 256], BF16, tag="x")
                        load_x(x0_sb, 0, 0, first=True)
                        nc.sync.dma_start(wq_sb[:, cB], wq_d[:, cB])
                        c0 = slice(0, 8)
                        nc.sync.dma_start(wk_sb[:, c0], wk_d[:, c0])
                        nc.sync.dma_start(cs_sb, cs_d)
                        nc.sync.dma_start(sn_sb, sn_d)
                        nc.sync.dma_start(wv_sb[:, c0], wv_d[:, c0])
                        for ck in range(1, 4):
                            cks = slice(ck * 8, (ck + 1) * 8)
                            nc.sync.dma_start(wq_sb[:, cks], wq_d[:, cks])
                            nc.sync.dma_start(wk_sb[:, cks], wk_d[:, cks])
                            nc.sync.dma_start(wv_sb[:, cks], wv_d[:, cks])

                        # sweep order: both batches' half 0 first (only the
                        # first AllGather is needed), then half 1; batch 0's
                        # half 1 before batch 1's so phase B's first batch
                        # is fully staged as early as possible.
                        for b, half in ((0, 0), (1, 0), (0, 1), (1, 1)):
                          for tc_i in range(b * (NT // B),
                                            (b + 1) * (NT // B)):
                            if tc_i == 0 and half == 0:
                                x_sb = x0_sb
                            else:
                                x_sb = xp.tile([128, 32, 256], BF16, tag="x")
                                load_x(x_sb, half, tc_i)
                            qstg = sp.tile([128, HPC, 256], BF16, tag="qs")
                            kstg = sp.tile([128, HPC, 256], BF16, tag="ks")
                            for ts2 in range(2):
                                ts = half * 2 + ts2
                                # position subtile index within the batch
                                ps_i = (tc_i % (NT // B)) * 4 + ts
                                lhs = x_sb[:, :, ts2 * 128:(ts2 + 1) * 128]
                                # --- Q and K with rope ---
                                for w_sb, stg in ((wq_sb, qstg),
                                                  (wk_sb, kstg)):
                                    ps = pp.tile([128, DPC], F32, tag="qk")
                                    for kt in range(32):
                                        nc.tensor.matmul(
                                            ps, lhs[:, kt], w_sb[:, kt],
                                            start=(kt == 0), stop=(kt == 31))
                                    c_ap = cs_sb[:, ps_i]
                                    s_ap = sn_sb[:, ps_i]
                                    pr = ps[:, 0:256]
                                    pi = ps[:, 256:512]
                                    t_rc = rtp.tile([128, 256], F32, tag="t0")
                                    t_is = rtp.tile([128, 256], F32, tag="t1")
                                    t_rs = rtp.tile([128, 256], F32, tag="t2")
                                    t_ic = rtp.tile([128, 256], F32, tag="t3")
                                    ro = rtp.tile([128, HPC, 128], BF16,
                                                  tag="ro")
                                    nc.vector.tensor_mul(t_rc, pr, c_ap)
                                    nc.vector.tensor_mul(t_is, pi, s_ap)
                                    nc.vector.tensor_mul(t_rs, pr, s_ap)
                                    nc.vector.tensor_mul(t_ic, pi, c_ap)
                                    nc.vector.tensor_sub(
                                        ro[:, :, 0:64], t_rc, t_is)
                                    nc.vector.tensor_add(
                                        ro[:, :, 64:128], t_rs, t_ic)
                                    for h in range(HPC):
                                        tp = tpp.tile([128, 128], BF16,
                                                      tag="tp")
                                        nc.tensor.transpose(
                                            tp, ro[:, h], ident_bf)
                                        dst = stg[:, h,
                                                  ts2 * 128:(ts2 + 1) * 128]
                                        if h % 2 == 0:
                                            nc.scalar.copy(dst, tp)
                                        else:
                                            nc.vector.tensor_copy(dst, tp)
                                # --- V (no rope) ---
                                vps = vpp.tile([128, DPC], F32, tag="v")
                                for kt in range(32):
                                    nc.tensor.matmul(
                                        vps, lhs[:, kt], wv_sb[:, kt],
                                        start=(kt == 0), stop=(kt == 31))
                                v_sb = sp.tile([128, DPC], BF16, tag="vo")
                                nc.scalar.copy(v_sb, vps)
                                tt = (tc_i % (NT // B)) * 4 + ts
                                nc.sync.dma_start(v_d[b, tt], v_sb)
                            nc.sync.dma_start(
                                qt_d[tc_i][:, :,
                                           half * 256:(half + 1) * 256],
                                qstg)
                            nc.sync.dma_start(
                                kt_d[tc_i][:, :,
                                           half * 256:(half + 1) * 256],
                                kstg)

                # ---------------- Phase B: attention -------------------------
                with tc.tile_pool(name="bres", bufs=1) as brp:
                    ot_res = brp.tile([128, B, HPC, S], BF16, tag="ot")
                    wo_sb = brp.tile([128, HPC, D], BF16, tag="wo")
                    if "b" not in phases:
                        nc.sync.dma_start(wo_sb, wo_d)
                    if "b" in phases:
                        with tc.tile_pool(name="bmask", bufs=1) as mp, \
                             tc.tile_pool(name="bv", bufs=2) as vp, \
                             tc.tile_pool(name="bqk", bufs=2) as qkp, \
                             tc.tile_pool(name="bp", bufs=4) as ptp, \
                             tc.tile_pool(name="bz", bufs=2) as zp, \
                             tc.tile_pool(name="bsps", bufs=4,
                                          space="PSUM") as spsp, \
                             tc.tile_pool(name="bops", bufs=2,
                                          space="PSUM") as opsp, \
                             tc.tile_pool(name="bzps", bufs=2,
                                          space="PSUM") as zpsp:
                            if causal:
                                mb_sb = mp.tile([128, 4, 512], BF16,
                                                tag="mb")
                            else:
                                mb_sb = mp.tile([128, S // 128, S], BF16,
                                                tag="mb")
                            nc.sync.dma_start(mb_sb, mb_d)
                            for b in range(B):
                                vt_sb = vp.tile([128, S // 128, DPC], BF16,
                                                tag="vt")
                                for h in range(HPC):
                                    qt_sb = qkp.tile([128, QG, 512], BF16,
                                                     tag="qt")
                                    kt_sb = qkp.tile([128, QG, 512], BF16,
                                                     tag="kt")
                                    nc.sync.dma_start(
                                        qt_sb,
                                        qt_d[b * QG:(b + 1) * QG, :, h]
                                        .rearrange("t p c -> p t c"))
                                    nc.sync.dma_start(
                                        kt_sb,
                                        kt_d[b * QG:(b + 1) * QG, :, h]
                                        .rearrange("t p c -> p t c"))
                                    if h == 0:
                                        # V queued behind the first head's
                                        # Q^T/K^T so scores start sooner;
                                        # PV needs V only after the first exp
                                        nc.sync.dma_start(
                                            vt_sb,
                                            v_d[b].rearrange(
                                                "tt p m -> p tt m"))
                                    if b == 0 and h == 0:
                                        # wo (phase C) queued behind the
                                        # first attention loads
                                        nc.sync.dma_start(wo_sb, wo_d)
                                    for qg in range(QG):
                                        nkt = (qg + 1) * 4 if causal \
                                            else S // 128
                                        ops = opsp.tile([128, 512], F32,
                                                        tag="o")
                                        zrz = zpsp.tile([128, 512], F32,
                                                        tag="z")
                                        for kt in range(nkt):
                                            sps = spsp.tile([128, 512], F32,
                                                            tag="s")
                                            diag = kt - qg * 4
                                            if causal:
                                                m_ap = None
                                            else:
                                                m_ap = mb_sb[
                                                    :, kt,
                                                    qg * 512:(qg + 1) * 512]
                                            if m_ap is not None:
                                                nc.tensor.matmul(
                                                    sps, ident_bf, m_ap,
                                                    start=True, stop=False)
                                            nc.tensor.matmul(
                                                sps,
                                                kt_sb[:, kt // 4,
                                                      (kt % 4) * 128:
                                                      (kt % 4 + 1) * 128],
                                                qt_sb[:, qg],
                                                start=(m_ap is None),
                                                stop=True)
                                            pt_sb = ptp.tile([128, 512], BF16,
                                                             tag="pt")
                                            nc.scalar.activation(
                                                pt_sb, sps,
                                                mybir.ActivationFunctionType
                                                .Exp)
                                            if causal and diag >= 0:
                                                # zero the upper-triangle
                                                # band post-exp on DVE
                                                nc.vector.tensor_mul(
                                                    pt_sb, pt_sb,
                                                    mb_sb[:, diag])
                                            nc.tensor.matmul(
                                                ops,
                                                vt_sb[:, kt,
                                                      h * 128:(h + 1) * 128],
                                                pt_sb,
                                                start=(kt == 0),
                                                stop=(kt == nkt - 1))
                                            nc.tensor.matmul(
                                                zrz[0:1], ones_bf[:, 0:1],
                                                pt_sb,
                                                start=(kt == 0),
                                                stop=(kt == nkt - 1))
                                        z_sb = zp.tile([128, 512], F32,
                                                       tag="zf")
                                        rz_sb = zp.tile([128, 512], BF16,
                                                        tag="rz")
                                        nc.vector.reciprocal(
                                            z_sb[0:1], zrz[0:1])
                                        nc.vector.tensor_copy(
                                            rz_sb[0:1], z_sb[0:1])
                                        nc.tensor.matmul(
                                            zrz, ones_bf[0:1],
                                            rz_sb[0:1],
                                            start=True, stop=True)
                                        rzb_sb = zp.tile([128, 512], BF16,
                                                         tag="rzb")
                                        nc.scalar.copy(rzb_sb, zrz)
                                        nc.vector.tensor_mul(
                                            ot_res[:, b, h,
                                                   qg * 512:(qg + 1) * 512],
                                            ops, rzb_sb)

                    # ---------------- Phase C: output projection -------------
                    if "c" in phases:
                        with tc.tile_pool(name="co", bufs=3) as cop, \
                             tc.tile_pool(name="cps", bufs=4,
                                          space="PSUM") as cpp:
                            tt_per_chunk = (T // 128) // RS_CHUNKS
                            for tt in range(T // 128):
                                b = tt // (S // 128)
                                qg = (tt % (S // 128)) // 4
                                off = (tt % 4) * 128
                                o_sb = cop.tile([128, D], ODT, tag="os")
                                for dc in range(8):
                                    ps = cpp.tile([128, 512], F32, tag="cps")
                                    for h in range(HPC):
                                        nc.tensor.matmul(
                                            ps,
                                            ot_res[:, b, h,
                                                   qg * 512 + off:
                                                   qg * 512 + off + 128],
                                            wo_sb[:, h,
                                                  dc * 512:(dc + 1) * 512],
                                            start=(h == 0), stop=(h == 3))
                                    if dc % 2 == 0:
                                        nc.scalar.copy(
                                            o_sb[:, dc * 512:(dc + 1) * 512],
                                            ps)
                                    else:
                                        nc.vector.tensor_copy(
                                            o_sb[:, dc * 512:(dc + 1) * 512],
                                            ps)
                                nc.sync.dma_start(
                                    part_d[tt * 128:(tt + 1) * 128, :], o_sb)
                                # reduce-scatter each chunk as soon as its
                                # 8 tt-tiles are written
                                if (tt + 1) % tt_per_chunk == 0:
                                    g = tt // tt_per_chunk
                                    rows = tt_per_chunk * 128
                                    nc.gpsimd.collective_compute(
                                        "ReduceScatter", mybir.AluOpType.add,
                                        replica_groups=GROUPS,
                                        ins=[part_d[g * rows:(g + 1) * rows]],
                                        outs=[rso_d[g]])
                                    nc.sync.dma_start(
                                        outs_d[g * (rows // NCORES):
                                               (g + 1) * (rows // NCORES)],
                                        rso_d[g])

    nc.compile()
    return nc


_NC_CACHE = {}


def _get_nc(cst_arr, snt_arr, mask2d=None):
    key = (mask2d is None,
           hashlib.sha256(cst_arr.tobytes()).hexdigest()[:16],
           hashlib.sha256(snt_arr.tobytes()).hexdigest()[:16],
           None if mask2d is None
           else hashlib.sha256(mask2d.tobytes()).hexdigest()[:16])
    if key not in _NC_CACHE:
        _NC_CACHE[key] = build_nc(cst_arr, snt_arr, mask2d=mask2d)
    return _NC_CACHE[key]


def _prep_inputs(x, wq, wk, wv, wo, freqs_cos, freqs_sin):
    bf = ml_dtypes.bfloat16
    xf = x.reshape(T, D)
    # x^T tiles split by token halves:
    # xt[tc, hf, p, kt, j] = x[tc*512 + hf*256 + j, kt*128 + p]
    xt = np.ascontiguousarray(
        xf.reshape(NT, 2, 256, 32, 128).transpose(0, 1, 4, 3, 2)).astype(bf)

    # per-core column permutation: within each core's 512 cols, all rope
    # "r" components (even hd) of the 4 heads first (h*64+j <- h*128+2j),
    # then all "i" components (odd hd)
    perm = np.empty(DPC, np.int64)
    for h in range(HPC):
        perm[h * 64:(h + 1) * 64] = h * 128 + 2 * np.arange(64)
        perm[256 + h * 64:256 + (h + 1) * 64] = h * 128 + 2 * np.arange(64) + 1

    qscale = 1.0 / math.sqrt(HD)

    def wtile(w):  # [D, DPC] -> [128, 32, DPC]
        return np.ascontiguousarray(
            w.reshape(32, 128, DPC).transpose(1, 0, 2)).astype(bf)

    in_maps = []
    for c in range(NCORES):
        cs = slice(c * DPC, (c + 1) * DPC)
        in_maps.append({
            "xtc": xt[c],
            "wq3": wtile(np.ascontiguousarray(wq[:, cs][:, perm]) * qscale),
            "wk3": wtile(np.ascontiguousarray(wk[:, cs][:, perm])),
            "wv3": wtile(np.ascontiguousarray(wv[:, cs])),
            "wo3": np.ascontiguousarray(
                wo[cs, :].reshape(HPC, 128, D).transpose(1, 0, 2)).astype(bf),
        })
    return in_maps


def _check_causal(mask2d):
    lower_ok = np.allclose(np.tril(mask2d), 0.0, atol=0.0)
    upper = mask2d[np.triu_indices(S, k=1)]
    upper_ok = upper.size == 0 or bool((upper <= -1e8).all())
    return lower_ok and upper_ok


def _assemble(results):
    """Undo the chunked reduce-scatter: core c's outs[g] block holds final
    rows [1024*g + 128*c : 1024*g + 128*(c+1)]."""
    out = np.empty((T, D), np.float32)
    rows_per_g = T // RS_CHUNKS          # 1024
    sub = rows_per_g // NCORES           # 128
    for c in range(NCORES):
        o = results[c]["outs"]
        for g in range(RS_CHUNKS):
            out[rows_per_g * g + sub * c:
                rows_per_g * g + sub * (c + 1)] = o[sub * g:sub * (g + 1)]
    return out


def kernel(x, wq, wk, wv, wo, freqs_cos, freqs_sin, mask, start_pos=0,
           _want_trace=False, **_ignored):
    x = np.asarray(x, dtype=np.float32)
    wq = np.asarray(wq, dtype=np.float32)
    wk = np.asarray(wk, dtype=np.float32)
    wv = np.asarray(wv, dtype=np.float32)
    wo = np.asarray(wo, dtype=np.float32)
    freqs_cos = np.asarray(freqs_cos, dtype=np.float32)
    freqs_sin = np.asarray(freqs_sin, dtype=np.float32)
    mask = np.asarray(mask, dtype=np.float32)
    mask2d = mask.reshape(S, S)
    causal = _check_causal(mask2d)

    cst_arr, snt_arr = _rope_tables(freqs_cos, freqs_sin)
    nc = _get_nc(cst_arr, snt_arr, mask2d=None if causal else mask2d)
    in_maps = _prep_inputs(x, wq, wk, wv, wo, freqs_cos, freqs_sin)
    res = run_bass_kernel_spmd(nc, in_maps, list(range(NCORES)),
                               trace=_want_trace)
    out = _assemble(res.results).reshape(B, S, D)
    if _want_trace:
        return out, res
    return out
